# revision 1
# baseline (speedup 1.0000x reference)
"""Chamfer-with-normals (6D NN search) Trainium2 kernel.

Strategy (8 NeuronCores, SPMD, no collectives):
  - 8 jobs = (batch b in 0..3) x (direction in {1,2}); core = 2*b + dir.
  - Each job is a full [8192 query x 8192 db] brute-force 6D NN search.
  - Pass A: PE matmul computes q[i,j] = -dist2 = 2*x.y - |x|^2 - |y|^2 via
    K=8 augmented vectors; DVE tensor_reduce(max) -> rowmax_i.
  - Interlude: PE transpose + ScalarE(-1) puts -rowmax back as row 8 of the
    query-side K-stationary matrix (roundtrip through DRAM).
  - Pass B: PE recomputes in transposed orientation with K=9:
    z[j,i] = q[i,j] - rowmax_i, ~0 at the argmin (same products in the same
    K order; only the PE's final extended-precision rounding differs).
    ScalarE Relu(BIG*z + 1) gives a near-one-hot mask; pay-stationary
    PE matmuls (pay^T @ mask, 4x column-tiled PSUM accumulators, 4 per
    bank) accumulate the selected db payload rows
    (xyz, normal, count channel) across all db blocks.
  - Host: divides payloads by the count channel (exact for single-match
    rows), computes per-row distances, normalization, sign-invariant normal
    metric, means. Rows whose count is outside [0.95, 1.05] (near-ties,
    large-ulp outliers) are recomputed exactly on the host (vectorized).
  - Both q and z matmuls run as 4 concurrent 32-row PE array tiles
    (tile_position row packing) with the K-vectors replicated into 4
    SBUF partition groups.

HW quirk handled: a PE LdWeights can carry at most ONE semaphore wait, so
matmuls that would need two waits are preceded by tiny 1x1 "touch" matmuls
that absorb one of the pending semaphore conditions.
"""

import sys

import numpy as np

for _p in ("/opt/trn_rl_repo", "/opt/pypackages"):
    if _p not in sys.path:
        sys.path.insert(0, _p)

B = 4
N = 8192  # queries per job
M = 8192  # database per job
P = 128
CH = 7  # payload channels: xyz(3), normal(3), count(1)
# Soft one-hot band: the PE accumulates the K-chain in extended precision
# with a single final rounding, so pass-B z = q - rowmax at the argmax lands
# within +-half-ulp(q) (~1e-7 for typical |q|<2, up to ~4e-6 for outliers)
# instead of exactly 0. With scale 1e5 (band 1e-5) the argmax weight is
# 1 +- ~0.015 and any entry whose distance gap exceeds 1e-5 gets weight 0.
# The host divides payloads by the count channel (exact for single-match
# rows) and falls back to an exact recompute when the count falls outside
# [0.95, 1.05] (near-ties, far-outlier rows with large ulp).
BIG = 1.0e5
EPS = 1e-12

_PROG_CACHE = {}


def _build_program(n, m, mode="full"):
    import concourse.bass as bass
    import concourse.tile as tile
    from concourse import mybir
    from concourse.masks import make_identity
    from concourse.tile_rust import add_dep_helper

    f32 = mybir.dt.float32
    nb = n // P  # query row blocks
    mb = m // P  # db row blocks
    n_chunks = n // 512
    m_chunks = m // 512

    nc = bass.Bass()
    ab_d = nc.dram_tensor("ab", [9, n + m], f32, kind="ExternalInput")
    pay_d = nc.dram_tensor("pay", [P, mb * CH], f32, kind="ExternalInput")
    acc_w = (n // 512 // 4) * 512  # col-tiled payload accumulator width
    out_d = nc.dram_tensor("tpay", [P, acc_w], f32, kind="ExternalOutput")
    rmx_d = nc.dram_tensor("rmx", [n], f32)

    with tile.TileContext(nc) as tc:
        with tc.tile_pool(name="singles", bufs=1) as singles:
            # ab data replicated into 4 partition groups (base 0/32/64/96)
            # so K=8/K=9 matmuls can run as 4 concurrent 32-row PE tiles.
            ab_sb = singles.tile([P, n + m], f32)
            pay_sb = singles.tile([P, mb * CH], f32)
            ident = singles.tile([P, P], f32)
            rowmax = singles.tile([P, nb], f32)
            acc_sb = singles.tile([P, max(acc_w, P)], f32)
            # rmx_sb aliases the (yet-unwritten) acc_sb tile to avoid a fresh
            # SBUF region whose zone tracking would pull in unrelated DMA sems
            rmx_sb = acc_sb[0:nb, 0:P]

            def pe_touch(touch, ap, base=0):
                return nc.tensor.matmul(
                    out=touch[0:1, 0:1],
                    lhsT=ap,
                    rhs=ap,
                    start=True,
                    stop=True,
                    tile_position=(base, 0),
                )

            make_identity(nc, ident[:])
            for r in range(4):
                nc.sync.dma_start(
                    out=ab_sb[32 * r : 32 * r + 9, :], in_=ab_d[:]
                )
            nc.sync.dma_start(out=pay_sb[:], in_=pay_d[:])

            def a_g(r):
                return ab_sb[32 * r : 32 * r + 9, 0:n]

            def b_g(r):
                return ab_sb[32 * r : 32 * r + 9, n : n + m]

            with tc.tile_pool(name="touchps", bufs=1, space="PSUM") as tp0:
                touch0 = tp0.tile([1, 1], f32, space="PSUM")
                # absorb the input-DMA and identity-memset sems on PE early
                pe_touch(touch0, pay_sb[0:1, 0:1])
                pe_touch(touch0, ident[0:1, 0:1])
                for r in range(4):
                    pe_touch(
                        touch0, ab_sb[32 * r : 32 * r + 9, 0:1], base=32 * r
                    )

            # ---------------- Pass A: row maxima of q ----------------
            with (
                tc.tile_pool(name="qps", bufs=2, space="PSUM") as qps,
                tc.tile_pool(name="rm", bufs=4) as rmpool,
            ):
                n_rounds = m_chunks // 4
                for ib in range(nb):
                    rm = rmpool.tile([P, n_rounds], f32)
                    for rnd in range(n_rounds):
                        q = qps.tile([P, 2048], f32, space="PSUM")
                        for r in range(4):
                            c = rnd * 4 + r
                            nc.tensor.matmul(
                                out=q[:, r * 512 : (r + 1) * 512],
                                lhsT=a_g(r)[0:8, ib * P : (ib + 1) * P],
                                rhs=b_g(r)[0:8, c * 512 : (c + 1) * 512],
                                start=True,
                                stop=True,
                                tile_position=(32 * r, 0),
                            )
                        nc.vector.tensor_reduce(
                            out=rm[:, rnd : rnd + 1],
                            in_=q[:, 0:2048],
                            axis=mybir.AxisListType.X,
                            op=mybir.AluOpType.max,
                        )
                    nc.vector.tensor_reduce(
                        out=rowmax[:, ib : ib + 1],
                        in_=rm[:, 0:n_rounds],
                        axis=mybir.AxisListType.X,
                        op=mybir.AluOpType.max,
                    )

            # transpose rowmax [P, nb] -> [nb, P], negate, roundtrip to
            # row 8 of every ab group in natural i order.
            with (
                tc.tile_pool(name="rmxps", bufs=1, space="PSUM") as rmxps,
                tc.tile_pool(name="touchp2", bufs=1, space="PSUM") as tp2,
            ):
                rmx_ps = rmxps.tile([nb, P], f32, space="PSUM")
                nc.tensor.transpose(
                    out=rmx_ps[:], in_=rowmax[:, 0:nb], identity=ident[:]
                )
                nc.scalar.activation(
                    out=rmx_sb[:],
                    in_=rmx_ps[:],
                    func=mybir.ActivationFunctionType.Copy,
                    scale=-1.0,
                )
                nc.sync.dma_start(
                    out=rmx_d[:].rearrange("(a b) -> a b", a=nb), in_=rmx_sb[:]
                )
                touch2 = tp2.tile([1, 1], f32, space="PSUM")
                row8_touches = []
                for r in range(4):
                    nc.sync.dma_start(
                        out=ab_sb[32 * r + 8 : 32 * r + 9, 0:n],
                        in_=rmx_d[None, :],
                    )
                    # absorb each group's row-8 DMA sem on PE (K=9 column
                    # overlapping row 8 at the group's base partition)
                    row8_touches.append(
                        pe_touch(
                            touch2,
                            ab_sb[32 * r : 32 * r + 9, 0:1],
                            base=32 * r,
                        )
                    )

            # ---------------- Pass B: mask + payload ----------------
            with (
                tc.tile_pool(name="zps", bufs=2, space="PSUM") as zps,
                tc.tile_pool(name="accps", bufs=1, space="PSUM") as accps,
                tc.tile_pool(name="mask", bufs=3) as maskpool,
            ):
                acc = accps.tile([P, acc_w], f32, space="PSUM")
                # absorb the acc-bank WAR handover on PE before the real
                # accumulation group opens (col-tiled like the payload mms)
                nc.tensor.matmul(
                    out=acc[0:1, 0:1],
                    lhsT=ab_sb[0:1, 0:1],
                    rhs=ab_sb[0:1, 0:1],
                    start=True,
                    stop=True,
                    tile_position=(0, 0),
                )
                # 2-chunk z rounds, double-buffered: ScalarE streams the
                # mask continuously instead of ping-ponging with PE (PE has
                # slack; ACT is the pass-B pacer).
                zb_rounds = n_chunks // 2
                _next_z_dep = {}
                for jb in range(mb if mode != "passA" else 0):
                    mask = maskpool.tile([P, n], f32)
                    for rnd in range(zb_rounds):
                        z = zps.tile([P, 1024], f32, space="PSUM")
                        for r in range(2):
                            c = rnd * 2 + r
                            zmm = nc.tensor.matmul(
                                out=z[:, r * 512 : (r + 1) * 512],
                                lhsT=b_g(r)[:, jb * P : (jb + 1) * P],
                                rhs=a_g(r)[:, c * 512 : (c + 1) * 512],
                                start=True,
                                stop=True,
                                tile_position=(32 * r, 0),
                            )
                            if jb == 0 and rnd == 0:
                                add_dep_helper(
                                    zmm.ins,
                                    row8_touches[r].ins,
                                    reason="order row8 sem absorber first",
                                )
                            if rnd == 0 and r == 0 and jb in _next_z_dep:
                                add_dep_helper(
                                    zmm.ins,
                                    _next_z_dep[jb].ins,
                                    reason="group col-tiled payload mms",
                                )
                        nc.scalar.activation(
                            out=mask[:, rnd * 1024 : (rnd + 1) * 1024],
                            in_=z[:, 0:1024],
                            func=mybir.ActivationFunctionType.Relu,
                            scale=BIG,
                            bias=1.0,
                        )
                    if mode == "nopay":
                        continue
                    pay_first = None
                    for c in range(n_chunks):
                        pp = 32 * (c % 4)
                        fo = (c // 4) * 512
                        pmm = nc.tensor.matmul(
                            out=acc[pp : pp + CH, fo : fo + 512],
                            lhsT=pay_sb[:, jb * CH : (jb + 1) * CH],
                            rhs=mask[:, c * 512 : (c + 1) * 512],
                            start=(jb == 0),
                            stop=(jb == mb - 1),
                            tile_position=(0, pp),
                        )
                        if pay_first is None:
                            pay_first = pmm
                    if jb + 1 < mb:
                        _next_z_dep[jb + 1] = pay_first

                if mode == "full":
                    nc.vector.tensor_copy(acc_sb[:, 0:acc_w], acc[:])
                    nc.sync.dma_start(out=out_d[:], in_=acc_sb[:, 0:acc_w])


    _strip_redundant_pe_waits(nc)
    return nc


def _strip_redundant_pe_waits(nc):
    """Drop transitively-redundant semaphore waits from PE instructions.

    A PE LdWeights can carry only ONE sync wait, but Tile's sem assignment
    is not transitively minimal: a matmul often gets both a PE self-wait
    (PSUM WAW) and a DVE/ACT wait (WAR) where the latter already implies the
    former (the consumer that frees the PSUM slot itself waited on the PE
    writes). Soundness: sem >= v means the instructions contributing the
    first v increments have *completed*, hence their own waits were
    satisfied, recursively.
    """
    f = nc.m.functions[0]
    insts = [ins for bb in f.blocks for ins in bb.instructions]
    k_of = {id(ins): k for k, ins in enumerate(insts)}

    sem_incs = {}  # sem id -> list of (cum_value, inst_idx)
    for k, ins in enumerate(insts):
        si = ins.sync_info
        if si is None:
            continue
        for up in si.on_update:
            if up.sync_type != "semaphore" or up.update_mode not in (
                "sem-inc",
                "sem-add-imm",
            ):
                continue
            lst = sem_incs.setdefault(up.id, [])
            prev = lst[-1][0] if lst else 0
            lst.append((prev + up.update_value, k))

    closure_memo = {}
    prefix_memo = {}  # sem id -> (built_upto_index, list of merged dicts)

    def merge(dst, src):
        for s, v in src.items():
            if dst.get(s, -1) < v:
                dst[s] = v

    def closure(k):
        # ticks guaranteed completed once instruction k has completed
        got = closure_memo.get(k)
        if got is not None:
            return got
        closure_memo[k] = {}  # cycle guard
        out = {}
        si = insts[k].sync_info
        if si is not None:
            for w in si.on_wait:
                if (
                    w.sync_type == "semaphore"
                    and w.wait_mode == "sem-ge-imm"
                    and w.wait_reg is None
                ):
                    merge(out, wait_implies(w.id, w.wait_value))
        closure_memo[k] = out
        return out

    def wait_implies(semid, v):
        out = {semid: v}
        lst = sem_incs.get(semid, [])
        # incremental prefix closures per sem (shared list updated in place
        # so reentrant calls see consistent partial data)
        if semid not in prefix_memo:
            prefix_memo[semid] = []
        prefs = prefix_memo[semid]
        while True:
            idx = len(prefs)
            if idx >= len(lst) or lst[idx][0] > v:
                break
            cum, j = lst[idx]
            cj = closure(j)  # may reenter and extend prefs (only below cum)
            if len(prefs) != idx:
                continue
            base = dict(prefs[-1]) if prefs else {}
            merge(base, cj)
            base[semid] = cum
            prefs.append(base)
        # largest prefix with cum <= v
        lo, hi = 0, len(lst)
        while lo < hi:
            mid = (lo + hi) // 2
            if lst[mid][0] <= v:
                lo = mid + 1
            else:
                hi = mid
        if lo > 0:
            merge(out, prefs[lo - 1])
        return out

    for attempt in range(3):
        closure_memo.clear()
        prefix_memo.clear()
        bad = _strip_pass(
            insts, sem_incs, merge, wait_implies, push_extras=(attempt == 2)
        )
        if not bad:
            return
    raise RuntimeError(
        f"instructions still have >1 sync wait after transitive "
        f"reduction: {bad[:5]} ({len(bad)} total)"
    )


def _strip_pass(insts, sem_incs, merge, wait_implies, push_extras):
    bad = []
    for k, ins in enumerate(insts):
        limit = 1
        si = ins.sync_info
        if si is None or len(si.on_wait) <= limit:
            continue
        waits = list(si.on_wait)
        changed = True
        while len(waits) > 1 and changed:
            changed = False
            for wi, w in enumerate(waits):
                if not (
                    w.sync_type == "semaphore"
                    and w.wait_mode == "sem-ge-imm"
                    and w.wait_reg is None
                ):
                    continue
                implied = {}
                for wj, w2 in enumerate(waits):
                    if wj == wi:
                        continue
                    if (
                        w2.sync_type == "semaphore"
                        and w2.wait_mode == "sem-ge-imm"
                        and w2.wait_reg is None
                    ):
                        merge(implied, wait_implies(w2.id, w2.wait_value))
                if implied.get(w.id, -1) >= w.wait_value:
                    waits.pop(wi)
                    changed = True
                    break
        if len(waits) > limit and push_extras:
            # Fallback: push extra waits onto earlier same-engine
            # instructions. Safe when every increment satisfying the wait
            # sits earlier in the (topologically ordered) schedule than the
            # target instruction, so the moved wait cannot deadlock.
            def last_incrementer_pos(w):
                lst = sem_incs.get(w.id, [])
                pos = -1
                for cum, j in lst:
                    if cum > w.wait_value:
                        break
                    pos = max(pos, j)
                return pos

            waits.sort(key=last_incrementer_pos)
            keep = waits[-limit:]
            extras = waits[:-limit]
            eng = ins.engine.name
            kprev = k - 1
            while extras and kprev >= 0:
                cand = insts[kprev]
                csi = cand.sync_info
                if (
                    cand.engine.name == eng
                    and csi is not None
                    and len(csi.on_wait) == 0
                ):
                    w = extras[-1]
                    if last_incrementer_pos(w) < kprev:
                        extras.pop()
                        csi.on_wait = [w]
                        cand.sync_info = csi
                kprev -= 1
            waits = extras + keep
        if len(waits) > limit:
            bad.append((ins.name, [(w.ant_name, w.wait_value) for w in waits]))
        if len(waits) != len(si.on_wait):
            si.on_wait = waits
            ins.sync_info = si
    return bad


def _get_program(n, m, mode="full"):
    key = (n, m, mode)
    if key not in _PROG_CACHE:
        _PROG_CACHE[key] = _build_program(n, m, mode)
    return _PROG_CACHE[key]


def _l2norm(x):
    nrm = np.sqrt((x * x).sum(axis=-1, keepdims=True))
    return x / np.maximum(nrm, EPS)


def _host_inputs(q6, qsq, db6, dbsq, pay_xyz, pay_n, n, m):
    ab = np.empty((9, n + m), np.float32)
    ab[0:6, 0:n] = q6.T
    ab[6, 0:n] = qsq
    ab[7, 0:n] = 1.0
    ab[8, 0:n] = 0.0
    ab[0:6, n:] = 2.0 * db6.T
    ab[6, n:] = -1.0
    ab[7, n:] = -dbsq
    ab[8, n:] = 1.0
    pay = np.concatenate(
        [pay_xyz, pay_n, np.ones((m, 1), np.float32)], axis=1
    ).astype(np.float32)
    payb = np.ascontiguousarray(
        pay.reshape(m // P, P, CH).transpose(1, 0, 2).reshape(P, (m // P) * CH)
    )
    return {"ab": np.ascontiguousarray(ab), "pay": payb}


_LAST_RUN_INFO = {}
_RUNNER_CACHE = {}


def _get_runner(n, m, n_cores, mode="full"):
    """Build (once) a persistent jitted SPMD executor for the program.

    Mirrors concourse.bass2jax.run_bass_via_pjrt's multi-core path but
    caches the jitted callable so repeat kernel() calls skip re-lowering.
    """
    key = (n, m, n_cores, mode)
    if key in _RUNNER_CACHE:
        return _RUNNER_CACHE[key]

    import jax
    from jax.experimental.shard_map import shard_map
    from jax.sharding import Mesh, PartitionSpec

    from concourse import bass2jax, mybir

    nc = _get_program(n, m, mode)
    bass2jax.install_neuronx_cc_hook()

    partition_name = (
        nc.partition_id_tensor.name if nc.partition_id_tensor else None
    )
    in_names, out_names, out_avals, zero_outs = [], [], [], []
    for alloc in nc.m.functions[0].allocations:
        if not isinstance(alloc, mybir.MemoryLocationSet):
            continue
        name = alloc.memorylocations[0].name
        if alloc.kind == "ExternalInput":
            if name != partition_name:
                in_names.append(name)
        elif alloc.kind == "ExternalOutput":
            out_names.append(name)
            shape = tuple(alloc.tensor_shape)
            dtype = mybir.dt.np(alloc.dtype)
            out_avals.append(jax.core.ShapedArray(shape, dtype))
            zero_outs.append(np.zeros(shape, dtype))
    n_params = len(in_names)
    n_outs = len(out_avals)
    in_names_all = list(in_names) + list(out_names)
    if partition_name is not None:
        in_names_all.append(partition_name)

    def _body(*args):
        operands = list(args)
        if partition_name is not None:
            operands.append(bass2jax.partition_id_tensor())
        outs = bass2jax._bass_exec_p.bind(
            *operands,
            out_avals=tuple(out_avals),
            in_names=tuple(in_names_all),
            out_names=tuple(out_names),
            lowering_input_output_aliases=(),
            sim_require_finite=True,
            sim_require_nnan=True,
            nc=nc,
        )
        return tuple(outs)

    donate = tuple(range(n_params, n_params + n_outs))
    devices = jax.devices()[:n_cores]
    mesh = Mesh(np.asarray(devices), ("core",))
    sharded = jax.jit(
        shard_map(
            _body,
            mesh=mesh,
            in_specs=(PartitionSpec("core"),) * (n_params + n_outs),
            out_specs=(PartitionSpec("core"),) * n_outs,
            check_rep=False,
        ),
        donate_argnums=donate,
        keep_unused=True,
    )

    runner = {
        "sharded": sharded,
        "in_names": in_names,
        "out_names": out_names,
        "out_avals": out_avals,
        "zero_outs": zero_outs,
        "n_cores": n_cores,
    }
    _RUNNER_CACHE[key] = runner
    return runner


def _run_jobs(in_maps, n, m, mode="full"):
    import time

    n_cores = len(in_maps)
    r = _get_runner(n, m, n_cores, mode)
    concat_in = [
        np.concatenate([m_[name] for m_ in in_maps], axis=0)
        for name in r["in_names"]
    ]
    concat_zeros = [
        np.zeros((n_cores * z.shape[0], *z.shape[1:]), z.dtype)
        for z in r["zero_outs"]
    ]
    t0 = time.time()
    out_arrs = r["sharded"](*concat_in, *concat_zeros)
    out_np = [np.asarray(a) for a in out_arrs]
    _LAST_RUN_INFO["exec_wall_ns"] = (time.time() - t0) * 1e9
    _LAST_RUN_INFO["exec_time_ns"] = None
    name_i = {name: i for i, name in enumerate(r["out_names"])}
    i = name_i["tpay"]
    av = r["out_avals"][i]
    per_core = out_np[i].reshape(n_cores, *av.shape)
    return [per_core[c] for c in range(n_cores)]


def kernel(xyz1, xyz2, normal_rebuild, normal_gt):
    xyz1 = np.asarray(xyz1, np.float32)
    xyz2 = np.asarray(xyz2, np.float32)
    normal_rebuild = np.asarray(normal_rebuild, np.float32)
    normal_gt = np.asarray(normal_gt, np.float32)
    b, n = xyz1.shape[0], xyz1.shape[1]
    m = xyz2.shape[1]

    n1 = _l2norm(normal_rebuild)
    n2 = _l2norm(normal_gt)
    p1 = np.concatenate([xyz1, n1], axis=2)
    p2 = np.concatenate([xyz2, n2], axis=2)
    sq1 = (p1 * p1).sum(axis=2)
    sq2 = (p2 * p2).sum(axis=2)

    jobs = []  # (q6, qsq, db6, dbsq, pay_xyz, pay_n, q_xyz, q_n)
    in_maps = []
    for core in range(2 * b):
        bi, d = core // 2, core % 2
        if d == 0:
            job = (p1[bi], sq1[bi], p2[bi], sq2[bi], xyz2[bi], n2[bi],
                   xyz1[bi], n1[bi])
        else:
            job = (p2[bi], sq2[bi], p1[bi], sq1[bi], xyz1[bi], n1[bi],
                   xyz2[bi], n2[bi])
        jobs.append(job)
        in_maps.append(_host_inputs(job[0], job[1], job[2], job[3],
                                    job[4], job[5], n, m))

    outs = _run_jobs(in_maps, n, m)

    xyz_sums = [0.0, 0.0]
    nrm_sums = [0.0, 0.0]
    counts = [0, 0]
    for core, raw in enumerate(outs):
        d = core % 2
        q6, qsq, db6, dbsq, pay_xyz, pay_n, q_xyz, q_n = jobs[core]
        t = np.empty((n, CH), np.float32)
        for c in range(n // 512):
            blk = raw[32 * (c % 4) : 32 * (c % 4) + CH,
                      (c // 4) * 512 : (c // 4) * 512 + 512]
            t[c * 512 : (c + 1) * 512, :] = blk.T
        cnt = t[:, 6]
        safe = np.where(np.abs(cnt) > 1e-6, cnt, 1.0)
        t_xyz = t[:, 0:3] / safe[:, None]
        t_n = t[:, 3:6] / safe[:, None]
        bad = np.nonzero(np.abs(cnt - 1.0) > 0.05)[0]
        if bad.size:
            # exact host fallback (vectorized): ties / out-of-band rows
            dbad = (
                qsq[bad][:, None]
                + dbsq[None, :]
                - 2.0 * (q6[bad] @ db6.T)
            )
            j = np.argmin(dbad, axis=1)
            t_xyz[bad] = pay_xyz[j]
            t_n[bad] = pay_n[j]
        xyz_d = ((q_xyz - t_xyz) ** 2).sum(axis=1)
        a = _l2norm(q_n)
        tn = _l2norm(t_n)
        nd = np.minimum(
            ((a - tn) ** 2).sum(axis=1), ((a + tn) ** 2).sum(axis=1)
        )
        xyz_sums[d] += float(xyz_d.sum())
        nrm_sums[d] += float(nd.sum())
        counts[d] += n

    xyz_out = xyz_sums[0] / counts[0] + xyz_sums[1] / counts[1]
    nrm_out = nrm_sums[0] / counts[0] + nrm_sums[1] / counts[1]
    return (np.float32(xyz_out), np.float32(nrm_out))



# revision 2
# speedup vs baseline: 1.0613x; 1.0613x over previous
"""Chamfer-with-normals (6D NN search) Trainium2 kernel, v3.

Device program (per core, SPMD over 8 cores, no collectives):
  - 8 jobs = (batch b in 0..3) x (direction in {1,2}); core = 2*b + dir.
  - q[i,j] = 2*q6_i.db6_j - |db6_j|^2 via K=7 fp16 PE matmuls (f32 psum),
    DVE copy to SBUF, DVE max/max_index -> top-1 db index per query row.
  - Output: [128, 64] uint16 index matrix (16 KB) per core.

Host/runner:
  - ships one fp16 [7, n+m] matrix per core; exact fp32 metric on host.
  - per-core shards are device_put from 8 threads (axon RPCs overlap when
    issued concurrently; serial shard uploads dominate the wall otherwise),
    then assembled with make_array_from_single_device_arrays.
  - the donated output buffer is the previous call's device-resident output
    (first call uploads zeros once); output shards are fetched in threads.
"""

import sys
from concurrent.futures import ThreadPoolExecutor

import numpy as np

for _p in ("/opt/trn_rl_repo", "/opt/pypackages"):
    if _p not in sys.path:
        sys.path.insert(0, _p)

B = 4
N = 8192  # queries per job
M = 8192  # database per job
P = 128
EPS = 1e-12

_PROG_CACHE = {}


def _build_program(n, m, nb_limit=None):
    import concourse.bass as bass
    import concourse.tile as tile
    from concourse import mybir

    f16 = mybir.dt.float16
    f32 = mybir.dt.float32
    u16 = mybir.dt.uint16
    nb = nb_limit if nb_limit is not None else n // P  # query row blocks
    K = 7

    nc = bass.Bass()
    ab_d = nc.dram_tensor("ab", [K, n + m], f16, kind="ExternalInput")
    idx_d = nc.dram_tensor("idx", [P, nb], u16, kind="ExternalOutput")

    with tile.TileContext(nc) as tc:
        with (
            tc.tile_pool(name="singles", bufs=1) as singles,
            tc.tile_pool(name="qrows", bufs=2) as qrows,
            tc.tile_pool(name="tops", bufs=4) as tops,
            tc.tile_pool(name="qps", bufs=2, space="PSUM") as qps,
        ):
            ab_sb = singles.tile([K, n + m], f16)
            idx_sb = singles.tile([P, nb], u16)
            nc.sync.dma_start(out=ab_sb[:], in_=ab_d[:])

            for ib in range(nb):
                qrow = qrows.tile([P, m], f32)
                for rnd in range(m // 2048):
                    q = qps.tile([P, 2048], f32, space="PSUM")
                    for r in range(4):
                        c = rnd * 4 + r
                        nc.tensor.matmul(
                            out=q[:, r * 512 : (r + 1) * 512],
                            lhsT=ab_sb[:, ib * P : (ib + 1) * P],
                            rhs=ab_sb[:, n + c * 512 : n + (c + 1) * 512],
                            start=True,
                            stop=True,
                        )
                    nc.vector.tensor_copy(
                        qrow[:, rnd * 2048 : (rnd + 1) * 2048], q[:]
                    )
                top_val = tops.tile([P, 8], f32)
                top_idx = tops.tile([P, 8], u16)
                nc.vector.max(top_val[:], qrow[:])
                nc.vector.max_index(top_idx[:], top_val[:], qrow[:])
                nc.vector.tensor_copy(idx_sb[:, ib : ib + 1], top_idx[:, 0:1])

            nc.sync.dma_start(out=idx_d[:], in_=idx_sb[:])

    _reduce_extra_waits(nc)
    return nc


def _reduce_extra_waits(nc):
    """Drop transitively-redundant semaphore waits (walrus codegen allows at
    most ONE sync wait per instruction).

    Sound closure over two in-order streams per instruction:
      - issue stream (engine queue): an instruction executes only after every
        earlier instruction on its engine executed, hence their waits held;
      - completion stream (engine for compute, DMA hw queue for DMAs): a
        semaphore floor s >= v implies every incrementer of s up to v
        completed, hence their waits held and earlier same-proc completions
        fired.
    """
    import sys as _sys

    f = nc.m.functions[0]
    insts = [ins for bb in f.blocks for ins in bb.instructions]
    n_ins = len(insts)
    _sys.setrecursionlimit(max(_sys.getrecursionlimit(), 50 * n_ins + 1000))

    def _upd(ins):
        si = ins.sync_info
        if si is None:
            return None
        for up in si.on_update:
            if up.sync_type == "semaphore" and up.update_mode in (
                "sem-inc",
                "sem-add-imm",
            ):
                return up
        return None

    def _waits(ins):
        si = ins.sync_info
        if si is None:
            return []
        return [
            w
            for w in si.on_wait
            if w.sync_type == "semaphore"
            and w.wait_mode == "sem-ge-imm"
            and w.wait_reg is None
        ]

    sem_incs = {}  # sem id -> list of (cum_value, inst_idx)
    prev_comp = [None] * n_ins
    prev_issue = [None] * n_ins
    last_comp, last_issue = {}, {}
    for k, ins in enumerate(insts):
        up = _upd(ins)
        if up is not None:
            lst = sem_incs.setdefault(up.id, [])
            prev = lst[-1][0] if lst else 0
            lst.append((prev + up.update_value, k))
            proc = ("sem", up.id)
        else:
            proc = ("eng", ins.engine.name)
        if proc in last_comp:
            prev_comp[k] = last_comp[proc]
        last_comp[proc] = k
        ekey = ins.engine.name
        if ekey in last_issue:
            prev_issue[k] = last_issue[ekey]
        last_issue[ekey] = k

    def merge(dst, src):
        for s, v in src.items():
            if dst.get(s, -1) < v:
                dst[s] = v

    issue_memo, comp_memo = {}, {}
    IN_PROGRESS = object()

    def issue_known(k):
        got = issue_memo.get(k)
        if got is IN_PROGRESS:
            return {}
        if got is not None:
            return got
        issue_memo[k] = IN_PROGRESS
        out = {}
        if prev_issue[k] is not None:
            merge(out, issue_known(prev_issue[k]))
        for w in _waits(insts[k]):
            if out.get(w.id, -1) < w.wait_value:
                out[w.id] = w.wait_value
            merge(out, floor_closure(w.id, w.wait_value))
        issue_memo[k] = out
        return out

    def completed(k):
        got = comp_memo.get(k)
        if got is IN_PROGRESS:
            return {}
        if got is not None:
            return got
        comp_memo[k] = IN_PROGRESS
        out = {}
        if prev_comp[k] is not None:
            merge(out, completed(prev_comp[k]))
        merge(out, issue_known(k))
        up = _upd(insts[k])
        if up is not None:
            lst = sem_incs[up.id]
            lo, hi = 0, len(lst)
            while lo < hi:
                mid = (lo + hi) // 2
                if lst[mid][1] <= k:
                    lo = mid + 1
                else:
                    hi = mid
            if lo > 0 and out.get(up.id, -1) < lst[lo - 1][0]:
                out[up.id] = lst[lo - 1][0]
        comp_memo[k] = out
        return out

    def floor_closure(semid, v):
        out = {semid: v}
        lst = sem_incs.get(semid, [])
        lo, hi = 0, len(lst)
        while lo < hi:
            mid = (lo + hi) // 2
            if lst[mid][0] <= v:
                lo = mid + 1
            else:
                hi = mid
        if lo > 0:
            merge(out, completed(lst[lo - 1][1]))
        return out

    bad = []
    for k, ins in enumerate(insts):
        si = ins.sync_info
        if si is None or len(si.on_wait) <= 1:
            continue
        waits = list(si.on_wait)
        changed = True
        while len(waits) > 1 and changed:
            changed = False
            for wi, w in enumerate(waits):
                if not (
                    w.sync_type == "semaphore"
                    and w.wait_mode == "sem-ge-imm"
                    and w.wait_reg is None
                ):
                    continue
                known = {}
                if prev_issue[k] is not None:
                    merge(known, issue_known(prev_issue[k]))
                for wj, w2 in enumerate(waits):
                    if wj == wi:
                        continue
                    if (
                        w2.sync_type == "semaphore"
                        and w2.wait_mode == "sem-ge-imm"
                        and w2.wait_reg is None
                    ):
                        if known.get(w2.id, -1) < w2.wait_value:
                            known[w2.id] = w2.wait_value
                        merge(known, floor_closure(w2.id, w2.wait_value))
                if known.get(w.id, -1) >= w.wait_value:
                    waits.pop(wi)
                    changed = True
                    break
        if len(waits) > 1:
            bad.append(
                (ins.name, [(w.ant_name, w.wait_value) for w in waits])
            )
        if len(waits) != len(si.on_wait):
            si.on_wait = waits
            ins.sync_info = si
    if bad:
        raise RuntimeError(
            f"instructions still have >1 sync wait after reduction: "
            f"{bad[:5]} ({len(bad)} total)"
        )


def _get_program(n, m, nb_limit=None):
    key = (n, m, nb_limit)
    if key not in _PROG_CACHE:
        _PROG_CACHE[key] = _build_program(n, m, nb_limit)
    return _PROG_CACHE[key]


def _l2norm(x):
    nrm = np.sqrt((x * x).sum(axis=-1, keepdims=True))
    return x / np.maximum(nrm, EPS)


def _host_inputs(q6, db6, dbsq, n, m):
    ab = np.empty((7, n + m), np.float16)
    ab[0:6, 0:n] = q6.T
    ab[6, 0:n] = 1.0
    ab[0:6, n:] = 2.0 * db6.T
    ab[6, n:] = -dbsq
    return {"ab": ab}


_LAST_RUN_INFO = {}
_RUNNER_CACHE = {}
_POOL = ThreadPoolExecutor(max_workers=8)


def _get_runner(n, m, n_cores):
    """Build (once) a persistent jitted SPMD executor for the program."""
    key = (n, m, n_cores)
    if key in _RUNNER_CACHE:
        return _RUNNER_CACHE[key]

    import jax
    from jax.experimental.shard_map import shard_map
    from jax.sharding import Mesh, NamedSharding, PartitionSpec

    from concourse import bass2jax, mybir

    nc = _get_program(n, m)
    bass2jax.install_neuronx_cc_hook()

    partition_name = (
        nc.partition_id_tensor.name if nc.partition_id_tensor else None
    )
    in_names, out_names, out_avals, zero_outs = [], [], [], []
    for alloc in nc.m.functions[0].allocations:
        if not isinstance(alloc, mybir.MemoryLocationSet):
            continue
        name = alloc.memorylocations[0].name
        if alloc.kind == "ExternalInput":
            if name != partition_name:
                in_names.append(name)
        elif alloc.kind == "ExternalOutput":
            out_names.append(name)
            shape = tuple(alloc.tensor_shape)
            dtype = mybir.dt.np(alloc.dtype)
            out_avals.append(jax.core.ShapedArray(shape, dtype))
            zero_outs.append(np.zeros(shape, dtype))
    n_params = len(in_names)
    n_outs = len(out_avals)
    in_names_all = list(in_names) + list(out_names)
    if partition_name is not None:
        in_names_all.append(partition_name)

    def _body(*args):
        operands = list(args)
        if partition_name is not None:
            operands.append(bass2jax.partition_id_tensor())
        outs = bass2jax._bass_exec_p.bind(
            *operands,
            out_avals=tuple(out_avals),
            in_names=tuple(in_names_all),
            out_names=tuple(out_names),
            lowering_input_output_aliases=(),
            sim_require_finite=True,
            sim_require_nnan=True,
            nc=nc,
        )
        return tuple(outs)

    donate = tuple(range(n_params, n_params + n_outs))
    devices = jax.devices()[:n_cores]
    mesh = Mesh(np.asarray(devices), ("core",))
    sharded = jax.jit(
        shard_map(
            _body,
            mesh=mesh,
            in_specs=(PartitionSpec("core"),) * (n_params + n_outs),
            out_specs=(PartitionSpec("core"),) * n_outs,
            check_rep=False,
        ),
        donate_argnums=donate,
        keep_unused=True,
    )

    runner = {
        "sharded": sharded,
        "in_names": in_names,
        "out_names": out_names,
        "out_avals": out_avals,
        "zero_outs": zero_outs,
        "n_cores": n_cores,
        "devices": devices,
        "sharding": NamedSharding(mesh, PartitionSpec("core")),
        "prev_outs": None,  # device-resident donation buffers
    }
    _RUNNER_CACHE[key] = runner
    return runner


def _run_jobs(in_maps, n, m):
    import time

    import jax

    n_cores = len(in_maps)
    r = _get_runner(n, m, n_cores)
    devices = r["devices"]

    t0 = time.time()
    # threaded per-shard upload (concurrent RPCs overlap through the tunnel)
    def put(c):
        return [
            jax.device_put(in_maps[c][name], devices[c])
            for name in r["in_names"]
        ]

    shard_lists = list(_POOL.map(put, range(n_cores)))
    global_ins = []
    for i, name in enumerate(r["in_names"]):
        shards = [shard_lists[c][i] for c in range(n_cores)]
        aval0 = shards[0].shape
        gshape = (n_cores * aval0[0],) + tuple(aval0[1:])
        global_ins.append(
            jax.make_array_from_single_device_arrays(
                gshape, r["sharding"], shards
            )
        )

    if r["prev_outs"] is None:
        donation = [
            jax.device_put(
                np.zeros((n_cores * z.shape[0], *z.shape[1:]), z.dtype),
                r["sharding"],
            )
            for z in r["zero_outs"]
        ]
    else:
        donation = r["prev_outs"]

    out_arrs = r["sharded"](*global_ins, *donation)
    out_arrs = list(out_arrs)
    r["prev_outs"] = out_arrs

    # threaded per-shard fetch
    def fetch(shard):
        return np.asarray(shard.data)

    outs_np = []
    for a in out_arrs:
        shards = sorted(
            a.addressable_shards, key=lambda s: s.device.id
        )
        parts = list(_POOL.map(fetch, shards))
        outs_np.append(parts)

    _LAST_RUN_INFO["exec_wall_ns"] = (time.time() - t0) * 1e9
    _LAST_RUN_INFO["exec_time_ns"] = None
    name_i = {name: i for i, name in enumerate(r["out_names"])}
    return outs_np[name_i["idx"]]


def kernel(xyz1, xyz2, normal_rebuild, normal_gt):
    xyz1 = np.asarray(xyz1, np.float32)
    xyz2 = np.asarray(xyz2, np.float32)
    normal_rebuild = np.asarray(normal_rebuild, np.float32)
    normal_gt = np.asarray(normal_gt, np.float32)
    b, n = xyz1.shape[0], xyz1.shape[1]
    m = xyz2.shape[1]

    n1 = _l2norm(normal_rebuild)
    n2 = _l2norm(normal_gt)
    p1 = np.concatenate([xyz1, n1], axis=2)
    p2 = np.concatenate([xyz2, n2], axis=2)
    sq1 = (p1 * p1).sum(axis=2)
    sq2 = (p2 * p2).sum(axis=2)

    in_maps = []
    for core in range(2 * b):
        bi, d = core // 2, core % 2
        if d == 0:
            in_maps.append(_host_inputs(p1[bi], p2[bi], sq2[bi], n, m))
        else:
            in_maps.append(_host_inputs(p2[bi], p1[bi], sq1[bi], n, m))

    outs = _run_jobs(in_maps, n, m)

    xyz_sums = [0.0, 0.0]
    nrm_sums = [0.0, 0.0]
    counts = [0, 0]
    for core, idx in enumerate(outs):
        bi, d = core // 2, core % 2
        if d == 0:
            q_xyz, q_n = xyz1[bi], n1[bi]
            db_xyz, db_n = xyz2[bi], n2[bi]
        else:
            q_xyz, q_n = xyz2[bi], n2[bi]
            db_xyz, db_n = xyz1[bi], n1[bi]
        # idx[p, ib] = argmin for query row ib*128 + p
        j = idx.T.ravel().astype(np.int64)
        t_xyz = db_xyz[j]
        t_n = db_n[j]
        xyz_d = ((q_xyz - t_xyz) ** 2).sum(axis=1)
        nd = np.minimum(
            ((q_n - t_n) ** 2).sum(axis=1), ((q_n + t_n) ** 2).sum(axis=1)
        )
        xyz_sums[d] += float(xyz_d.sum())
        nrm_sums[d] += float(nd.sum())
        counts[d] += n

    xyz_out = xyz_sums[0] / counts[0] + xyz_sums[1] / counts[1]
    nrm_out = nrm_sums[0] / counts[0] + nrm_sums[1] / counts[1]
    return (np.float32(xyz_out), np.float32(nrm_out))
